# revision 12
# baseline (speedup 1.0000x reference)
"""Expert-parallel MoE routing kernel for Trainium2 (8 NeuronCores).

Problem: top-k(=2) softmax-gated MoE FFN (relu), followed by
log_softmax(sum(moe_out, axis=-1)) over the sequence dim.

Key algebraic observation: the graded output is
    log_softmax_S( sum_d moe_out[t, d] )
and
    sum_d moe_out[t, :] = sum_e g[t,e] * (relu(x_t @ W1_e + b1_e) @ rowsum(W2_e) + sum(b2_e))
so the entire second expert matmul collapses to a matvec against
s_e = rowsum(W2_e), which the host computes once (a single pass over w2).
The gate values are folded into the dispatched tokens on the host
(g * relu(x@W1) == relu((g*x)@W1) for g > 0 when b1 == 0).

Per-core device program (core e owns expert e):
  PE : h_pre = xtg^T @ W1  (bf16 operands, 256 matmuls of ~276 cols)
  ACT: relu(h_pre + b1) per [128, ln] m-tile, PSUM -> SBUF
  DVE: acc += relu_h * s_m  (signed per-partition scale, fused mult-add)
  PE : z = ones^T @ acc     (final 128-partition reduction, 1 matmul/chunk)
Host gathers z per core, scatter-adds into [T], applies log_softmax.

HBM traffic per core is ~5.4 MB (bf16 w1 + bf16 gate-scaled tokens);
w2 (8 MB/core in the baseline) never touches the device.
"""

import numpy as np

N_CORES = 8
P = 128
GRP = 2  # m-tiles per w1 column-group (one 256-col block per (group, kd))


def _round_up(v, m):
    return ((v + m - 1) // m) * m


_BUILD_CACHE = {}


def _build_program(D, H, ln, NC):
    """Trace + compile the single-core program (SPMD across 8 cores).

    Per-core inputs:
      xtg [P, KD*NC*ln] bf16  gate-scaled gathered tokens; block (kd, c) at
                              cols (kd*NC+c)*ln is xg[kd*P:(kd+1)*P, c*ln:(c+1)*ln]
      w1  [P, KD*H]     bf16  expert's first-layer weight; block (g, kd) at
                              cols (g*KD+kd)*GP is w1[kd*P:(kd+1)*P, g*GP:(g+1)*GP]
      sm  [P, 2*MH+1]   f32   cols [0:MH) = w2 row-sums (col m = s[m*P:(m+1)*P]),
                              [MH:2MH) = b1 tiled the same way, [2MH] = ones
    Output:
      z [1, NC*ln] f32  z[c] = sum_h s_h * relu(x_c @ w1_h + b1_h)
    """
    key = (D, H, ln, NC)
    if key in _BUILD_CACHE:
        return _BUILD_CACHE[key]

    import concourse.tile as tile
    from concourse import bacc, mybir

    f32 = mybir.dt.float32
    f32r = mybir.dt.float32r
    bf16 = mybir.dt.bfloat16
    KD = D // P   # k-tiles over D
    MH = H // P   # m-tiles over H
    NG = MH // GRP  # w1 column groups
    GP = GRP * P  # columns per w1 block

    nc = bacc.Bacc("TRN2", target_bir_lowering=False, debug=False)
    xtg_d = nc.dram_tensor("xtg", [P, KD * NC * ln], bf16, kind="ExternalInput").ap()
    w1_d = nc.dram_tensor("w1", [P, KD * H], bf16, kind="ExternalInput").ap()
    sm_d = nc.dram_tensor("sm", [P, 2 * MH + 1], f32, kind="ExternalInput").ap()
    z_d = nc.dram_tensor("z", [1, NC * ln], f32, kind="ExternalOutput").ap()

    with tile.TileContext(nc) as tc:
        with (
            tc.tile_pool(name="persist", bufs=1) as persist,
            tc.tile_pool(name="ht", bufs=6) as htp,
            tc.tile_pool(name="psum_h", bufs=6, space="PSUM") as psum_h,
            tc.tile_pool(name="psum_z", bufs=2, space="PSUM") as psum_z,
        ):
            # --- small loads first: w2 row-sums / b1 / ones ---
            sm_sb = persist.tile([P, 2 * MH + 1], f32)
            nc.sync.dma_start(out=sm_sb[:], in_=sm_d[:])
            w2s = sm_sb[:, 0:MH]
            b1t = sm_sb[:, MH : 2 * MH]
            ones = persist.tile([P, 1], f32r)
            nc.vector.tensor_copy(out=ones[:], in_=sm_sb[:, 2 * MH : 2 * MH + 1])

            # acc tiles — f32r so the final PE matvec accepts them; two
            # independent accumulation chains (even/odd m) per chunk halve
            # the serial DVE tail.  g == 0 writes them fresh (no memset:
            # walrus rejects f32r memset).
            acc = [
                [
                    persist.tile([P, ln], f32r, tag=f"acc{c}_{p}", name=f"acc{c}_{p}")
                    for p in range(GRP)
                ]
                for c in range(NC)
            ]

            # --- weights + activations, ordered for earliest PE start:
            # group 0's w1 kd-pieces interleaved with the xtg kd-pieces
            # (group 0 needs both; later groups only need their w1) ---
            xtg_sb = persist.tile([P, KD * NC * ln], bf16)
            w1_sb = persist.tile([P, KD * H], bf16)

            def xtg_load(kd0, nkd):
                sl = slice(kd0 * NC * ln, (kd0 + nkd) * NC * ln)
                nc.sync.dma_start(out=xtg_sb[:, sl], in_=xtg_d[:, sl])

            def w1_load(g, kd0, nkd):
                sl = slice((g * KD + kd0) * GP, (g * KD + kd0 + nkd) * GP)
                nc.sync.dma_start(out=w1_sb[:, sl], in_=w1_d[:, sl])

            # PE warmup: ~4.6us of dependency-free matmuls on scratch data
            # run during the fixed ~7us program prologue, flipping the HAM
            # clock-gate to 2.4GHz before the real stream starts (saves the
            # ~4us half-clock ramp the stream would otherwise pay)
            warm = persist.tile([P, ln], bf16, tag="warm", name="warm")
            nc.vector.memset(warm[:], 0.0)
            pwarm = psum_h.tile([P, ln], f32, tag="psh", name="psh")
            NWARM = 20
            for i in range(NWARM):
                nc.tensor.matmul(
                    pwarm[:],
                    warm[:, 0:P],
                    warm[:],
                    start=(i == 0),
                    stop=(i == NWARM - 1),
                    skip_group_check=True,
                )

            # each dma_start costs ~650ns of serial HWDGE issue time and
            # ~1.5-2us completion latency, and transfers drain FIFO; lead
            # with two small pieces so the first matmuls' data lands early,
            # then stream the rest in big pieces
            h = KD // 2
            xtg_load(0, 1)
            w1_load(0, 0, 1)
            xtg_load(1, h - 1)
            w1_load(0, 1, h - 1)
            xtg_load(h, KD - h)
            w1_load(0, h, KD - h)
            w1_load(1, 0, h)
            w1_load(1, h, KD - h)
            for g in range(2, NG):
                w1_load(g, 0, KD)

            # --- mm1 + relu + scaled accumulate, group-major (chunk inner
            # so the w1 stream paces 2x ahead of the PE's consumption) ---
            for g in range(NG):
                pss = [
                    [
                        psum_h.tile([P, ln], f32, tag="psh", name="psh")
                        for _ in range(NC)
                    ]
                    for _ in range(GRP)
                ]
                for kd in range(KD):
                    base = (g * KD + kd) * GP
                    for mi in range(GRP):
                        for c in range(NC):
                            nc.tensor.matmul(
                                pss[mi][c][:],
                                w1_sb[:, base + mi * P : base + (mi + 1) * P],
                                xtg_sb[:, (kd * NC + c) * ln : (kd * NC + c + 1) * ln],
                                start=(kd == 0),
                                stop=(kd == KD - 1),
                                skip_group_check=True,
                            )
                for c in range(NC):  # chunk-major so chunk 0's output path
                    for mi in range(GRP):  # overlaps chunk 1's tail
                        m = g * GRP + mi
                        ht = htp.tile([P, ln], f32, tag="ht", name="ht")
                        nc.scalar.activation(
                            ht[:],
                            pss[mi][c][:],
                            mybir.ActivationFunctionType.Relu,
                            bias=b1t[:, m : m + 1],
                        )
                        if g == 0:
                            nc.vector.tensor_scalar(
                                out=acc[c][mi][:],
                                in0=ht[:],
                                scalar1=w2s[:, m : m + 1],
                                scalar2=None,
                                op0=mybir.AluOpType.mult,
                            )
                        else:
                            nc.vector.scalar_tensor_tensor(
                                out=acc[c][mi][:],
                                in0=ht[:],
                                scalar=w2s[:, m : m + 1],
                                in1=acc[c][mi][:],
                                op0=mybir.AluOpType.mult,
                                op1=mybir.AluOpType.add,
                            )

            # --- final partition reduction + output ---
            z_sb = persist.tile([1, NC * ln], f32)
            for c in range(NC):
                pz = psum_z.tile([1, ln], f32, tag="psz", name="psz")
                for p in range(GRP):
                    nc.tensor.matmul(
                        pz[:],
                        ones[:],
                        acc[c][p][:],
                        start=(p == 0),
                        stop=(p == GRP - 1),
                        skip_group_check=True,
                    )
                nc.scalar.activation(
                    z_sb[:, c * ln : (c + 1) * ln],
                    pz[:],
                    mybir.ActivationFunctionType.Copy,
                    bias=0.0,
                )
            nc.sync.dma_start(out=z_d[:], in_=z_sb[:])

    nc.compile()
    _BUILD_CACHE[key] = nc
    return nc


def kernel(x, wg, w1, b1, w2, b2, k):
    import ml_dtypes
    from concourse.bass_utils import run_bass_kernel_spmd

    bf16 = ml_dtypes.bfloat16
    x = np.asarray(x)
    wg = np.asarray(wg)
    w1 = np.asarray(w1)
    b1 = np.asarray(b1)
    w2 = np.asarray(w2)
    b2 = np.asarray(b2)
    k = int(k)

    B, S, D = x.shape
    E = wg.shape[1]
    H = w1.shape[2]
    T = B * S
    KD = D // P
    MH = H // P
    assert E == N_CORES, f"expert-parallel layout assumes E == 8, got {E}"
    assert D % P == 0 and H % P == 0 and MH % GRP == 0, (D, H)

    xf = np.ascontiguousarray(x.reshape(T, D), dtype=np.float32)

    # --- gate + top-k routing (host; needed to build the dispatch shards) ---
    logits = xf @ wg.astype(np.float32)
    logits -= logits.max(axis=1, keepdims=True)
    np.exp(logits, out=logits)
    scores = logits / logits.sum(axis=1, keepdims=True)
    if k >= E:
        topi = np.broadcast_to(np.arange(E, dtype=np.int64), (T, E))
    else:
        topi = np.argpartition(-scores, k, axis=1)[:, :k]
    rows = np.arange(T)[:, None]
    topv = scores[rows, topi]

    # per-expert token lists
    idx_e = []
    val_e = []
    for e in range(E):
        tmask, kpos = np.nonzero(topi == e)
        idx_e.append(tmask)
        val_e.append(topv[tmask, kpos].astype(np.float32))
    max_cnt = max(len(i) for i in idx_e)

    # chunk geometry: NC chunks of ln <= 512 tokens (PSUM bank limit)
    NC = max(1, -(-max_cnt // 512))
    ln = _round_up(-(-max_cnt // NC), 4)
    C = NC * ln

    fold_gate = not b1.any()  # g*relu(u) == relu(g*u) only when b1 == 0
    s_e = w2.astype(np.float32).sum(axis=2)          # [E, H] row-sums
    b2s_e = b2.astype(np.float32).sum(axis=1)        # [E]

    nc = _build_program(D, H, ln, NC)

    in_maps = []
    for e in range(E):
        n_e = len(idx_e[e])
        xg = np.zeros((D, C), dtype=np.float32)
        if n_e:
            cols = xf[idx_e[e]].T
            if fold_gate:
                cols = cols * val_e[e][None, :]
            xg[:, :n_e] = cols
        # -> [P, KD*NC*ln], block (kd, c)
        xtg = np.ascontiguousarray(
            xg.reshape(KD, P, NC, ln).transpose(1, 0, 2, 3).reshape(P, KD * NC * ln)
        ).astype(bf16)
        # w1 -> [P, KD*H], block (g, kd) of GRP*P columns
        w1r = np.ascontiguousarray(
            w1[e]
            .astype(np.float32)
            .reshape(KD, P, MH // GRP, GRP * P)
            .transpose(1, 2, 0, 3)
            .reshape(P, KD * H)
        ).astype(bf16)
        sm = np.empty((P, 2 * MH + 1), dtype=np.float32)
        sm[:, 0:MH] = s_e[e].reshape(MH, P).T
        sm[:, MH : 2 * MH] = b1[e].astype(np.float32).reshape(MH, P).T
        sm[:, 2 * MH] = 1.0
        in_maps.append({"xtg": xtg, "w1": w1r, "sm": sm})

    res = run_bass_kernel_spmd(nc, in_maps, core_ids=list(range(N_CORES)))

    # --- combine: scatter-add per-(token, expert) scalars, then log_softmax ---
    s = np.zeros(T, dtype=np.float32)
    for e in range(E):
        n_e = len(idx_e[e])
        if n_e:
            z = res.results[e]["z"][0, :n_e].astype(np.float32)
            if fold_gate:
                s[idx_e[e]] += z
            else:
                s[idx_e[e]] += val_e[e] * z
    if b2s_e.any():
        for e in range(E):
            if len(idx_e[e]):
                s[idx_e[e]] += val_e[e] * b2s_e[e]

    sm = s.reshape(B, S)
    sm = sm - sm.max(axis=1, keepdims=True)
    out = sm - np.log(np.exp(sm).sum(axis=1, keepdims=True))
    return out.astype(np.float32)


# revision 14
# speedup vs baseline: 1.1628x; 1.1628x over previous
"""Expert-parallel MoE routing kernel for Trainium2 (8 NeuronCores).

Problem: top-k(=2) softmax-gated MoE FFN (relu), followed by
log_softmax(sum(moe_out, axis=-1)) over the sequence dim.

Key algebraic observation: the graded output is
    log_softmax_S( sum_d moe_out[t, d] )
and
    sum_d moe_out[t, :] = sum_e g[t,e] * (relu(x_t @ W1_e + b1_e) @ rowsum(W2_e) + sum(b2_e))
so the entire second expert matmul collapses to a matvec against
s_e = rowsum(W2_e), which the host computes once (a single pass over w2).
The gate values are folded into the dispatched tokens on the host
(g * relu(x@W1) == relu((g*x)@W1) for g > 0 when b1 == 0).

Per-core device program (core e owns expert e):
  PE : h_pre = xtg^T @ W1  (bf16 operands, 256 matmuls of ~276 cols)
  ACT: relu(h_pre + b1) per [128, ln] m-tile, PSUM -> SBUF
  DVE: acc += relu_h * s_m  (signed per-partition scale, fused mult-add)
  PE : z = ones^T @ acc     (final 128-partition reduction, 1 matmul/chunk)
Host gathers z per core, scatter-adds into [T], applies log_softmax.

HBM traffic per core is ~5.4 MB (bf16 w1 + bf16 gate-scaled tokens);
w2 (8 MB/core in the baseline) never touches the device.
"""

import numpy as np

N_CORES = 8
P = 128
GRP = 2  # m-tiles per w1 column-group (one 256-col block per (group, kd))


def _round_up(v, m):
    return ((v + m - 1) // m) * m


_BUILD_CACHE = {}


def _build_program(D, H, ln, NC):
    """Trace + compile the single-core program (SPMD across 8 cores).

    Per-core inputs:
      xtg [P, KD*NC*ln] bf16  gate-scaled gathered tokens; block (kd, c) at
                              cols (kd*NC+c)*ln is xg[kd*P:(kd+1)*P, c*ln:(c+1)*ln]
      w1  [P, KD*H]     bf16  expert's first-layer weight; block (g, kd) at
                              cols (g*KD+kd)*GP is w1[kd*P:(kd+1)*P, g*GP:(g+1)*GP]
      sm  [P, 2*MH+1]   f32   cols [0:MH) = w2 row-sums (col m = s[m*P:(m+1)*P]),
                              [MH:2MH) = b1 tiled the same way, [2MH] = ones
    Output:
      z [1, NC*ln] f32  z[c] = sum_h s_h * relu(x_c @ w1_h + b1_h)
    """
    key = (D, H, ln, NC)
    if key in _BUILD_CACHE:
        return _BUILD_CACHE[key]

    import concourse.tile as tile
    from concourse import bacc, mybir

    f32 = mybir.dt.float32
    f32r = mybir.dt.float32r
    bf16 = mybir.dt.bfloat16
    KD = D // P   # k-tiles over D
    MH = H // P   # m-tiles over H
    NG = MH // GRP  # w1 column groups
    GP = GRP * P  # columns per w1 block

    nc = bacc.Bacc("TRN2", target_bir_lowering=False, debug=False)
    xtg_d = nc.dram_tensor("xtg", [P, KD * NC * ln], bf16, kind="ExternalInput").ap()
    w1_d = nc.dram_tensor("w1", [P, KD * H], bf16, kind="ExternalInput").ap()
    sm_d = nc.dram_tensor("sm", [P, 2 * MH + 1], f32, kind="ExternalInput").ap()
    z_d = nc.dram_tensor("z", [1, NC * ln], f32, kind="ExternalOutput").ap()

    with tile.TileContext(nc) as tc:
        with (
            tc.tile_pool(name="persist", bufs=1) as persist,
            tc.tile_pool(name="ht", bufs=6) as htp,
            tc.tile_pool(name="psum_h", bufs=6, space="PSUM") as psum_h,
            tc.tile_pool(name="psum_z", bufs=2, space="PSUM") as psum_z,
        ):
            # --- small loads first: w2 row-sums / b1 / ones ---
            sm_sb = persist.tile([P, 2 * MH + 1], f32)
            nc.sync.dma_start(out=sm_sb[:], in_=sm_d[:])
            w2s = sm_sb[:, 0:MH]
            b1t = sm_sb[:, MH : 2 * MH]
            ones = persist.tile([P, 1], f32r)
            nc.vector.tensor_copy(out=ones[:], in_=sm_sb[:, 2 * MH : 2 * MH + 1])

            # acc tiles — f32r so the final PE matvec accepts them; two
            # independent accumulation chains (even/odd m) per chunk halve
            # the serial DVE tail.  g == 0 writes them fresh (no memset:
            # walrus rejects f32r memset).
            acc = [
                [
                    persist.tile([P, ln], f32r, tag=f"acc{c}_{p}", name=f"acc{c}_{p}")
                    for p in range(GRP)
                ]
                for c in range(NC)
            ]

            # --- weights + activations, ordered for earliest PE start:
            # group 0's w1 kd-pieces interleaved with the xtg kd-pieces
            # (group 0 needs both; later groups only need their w1) ---
            xtg_sb = persist.tile([P, KD * NC * ln], bf16)
            w1_sb = persist.tile([P, KD * H], bf16)

            def xtg_load(kd0, nkd):
                sl = slice(kd0 * NC * ln, (kd0 + nkd) * NC * ln)
                nc.sync.dma_start(out=xtg_sb[:, sl], in_=xtg_d[:, sl])

            def w1_load(g, kd0, nkd):
                sl = slice((g * KD + kd0) * GP, (g * KD + kd0 + nkd) * GP)
                nc.sync.dma_start(out=w1_sb[:, sl], in_=w1_d[:, sl])

            # PE warmup: ~4.6us of dependency-free matmuls on scratch data
            # run during the fixed ~7us program prologue, flipping the HAM
            # clock-gate to 2.4GHz before the real stream starts (saves the
            # ~4us half-clock ramp the stream would otherwise pay)
            warm = persist.tile([P, ln], bf16, tag="warm", name="warm")
            nc.vector.memset(warm[:], 0.0)
            pwarm = psum_h.tile([P, ln], f32, tag="psh", name="psh")
            NWARM = 16
            for i in range(NWARM):
                nc.tensor.matmul(
                    pwarm[:],
                    warm[:, 0:P],
                    warm[:],
                    start=(i == 0),
                    stop=(i == NWARM - 1),
                    skip_group_check=True,
                )

            # each dma_start costs ~650ns of serial HWDGE issue time and
            # ~1.5-2us completion latency, and transfers drain FIFO; lead
            # with two small pieces so the first matmuls' data lands early,
            # then stream the rest in big pieces
            h = KD // 2
            xtg_load(0, 1)
            w1_load(0, 0, 1)
            xtg_load(1, h - 1)
            w1_load(0, 1, h - 1)
            xtg_load(h, KD - h)
            w1_load(0, h, KD - h)
            w1_load(1, 0, h)
            w1_load(1, h, KD - h)
            for g in range(2, NG):
                w1_load(g, 0, KD)

            # --- mm1 + relu + scaled accumulate, group-major (chunk inner
            # so the w1 stream paces 2x ahead of the PE's consumption) ---
            for g in range(NG):
                pss = [
                    [
                        psum_h.tile([P, ln], f32, tag="psh", name="psh")
                        for _ in range(NC)
                    ]
                    for _ in range(GRP)
                ]
                for kd in range(KD):
                    base = (g * KD + kd) * GP
                    for mi in range(GRP):
                        for c in range(NC):
                            nc.tensor.matmul(
                                pss[mi][c][:],
                                w1_sb[:, base + mi * P : base + (mi + 1) * P],
                                xtg_sb[:, (kd * NC + c) * ln : (kd * NC + c + 1) * ln],
                                start=(kd == 0),
                                stop=(kd == KD - 1),
                                skip_group_check=True,
                            )
                for c in range(NC):  # chunk-major so chunk 0's output path
                    for mi in range(GRP):  # overlaps chunk 1's tail
                        m = g * GRP + mi
                        ht = htp.tile([P, ln], f32, tag="ht", name="ht")
                        nc.scalar.activation(
                            ht[:],
                            pss[mi][c][:],
                            mybir.ActivationFunctionType.Relu,
                            bias=b1t[:, m : m + 1],
                        )
                        if g == 0:
                            nc.vector.tensor_scalar(
                                out=acc[c][mi][:],
                                in0=ht[:],
                                scalar1=w2s[:, m : m + 1],
                                scalar2=None,
                                op0=mybir.AluOpType.mult,
                            )
                        else:
                            nc.vector.scalar_tensor_tensor(
                                out=acc[c][mi][:],
                                in0=ht[:],
                                scalar=w2s[:, m : m + 1],
                                in1=acc[c][mi][:],
                                op0=mybir.AluOpType.mult,
                                op1=mybir.AluOpType.add,
                            )

            # --- final partition reduction + output (per-chunk DMA so chunk
            # 0's output overlaps chunk 1's accumulate tail) ---
            z_sb = persist.tile([1, NC * ln], f32)
            for c in range(NC):
                pz = psum_z.tile([1, ln], f32, tag="psz", name="psz")
                for p in range(GRP):
                    nc.tensor.matmul(
                        pz[:],
                        ones[:],
                        acc[c][p][:],
                        start=(p == 0),
                        stop=(p == GRP - 1),
                        skip_group_check=True,
                    )
                sl = slice(c * ln, (c + 1) * ln)
                nc.scalar.activation(
                    z_sb[:, sl],
                    pz[:],
                    mybir.ActivationFunctionType.Copy,
                    bias=0.0,
                )
                nc.sync.dma_start(out=z_d[:, sl], in_=z_sb[:, sl])

    nc.compile()
    _BUILD_CACHE[key] = nc
    return nc


def kernel(x, wg, w1, b1, w2, b2, k):
    import ml_dtypes
    from concourse.bass_utils import run_bass_kernel_spmd

    bf16 = ml_dtypes.bfloat16
    x = np.asarray(x)
    wg = np.asarray(wg)
    w1 = np.asarray(w1)
    b1 = np.asarray(b1)
    w2 = np.asarray(w2)
    b2 = np.asarray(b2)
    k = int(k)

    B, S, D = x.shape
    E = wg.shape[1]
    H = w1.shape[2]
    T = B * S
    KD = D // P
    MH = H // P
    assert E == N_CORES, f"expert-parallel layout assumes E == 8, got {E}"
    assert D % P == 0 and H % P == 0 and MH % GRP == 0, (D, H)

    xf = np.ascontiguousarray(x.reshape(T, D), dtype=np.float32)

    # --- gate + top-k routing (host; needed to build the dispatch shards) ---
    logits = xf @ wg.astype(np.float32)
    logits -= logits.max(axis=1, keepdims=True)
    np.exp(logits, out=logits)
    scores = logits / logits.sum(axis=1, keepdims=True)
    if k >= E:
        topi = np.broadcast_to(np.arange(E, dtype=np.int64), (T, E))
    else:
        topi = np.argpartition(-scores, k, axis=1)[:, :k]
    rows = np.arange(T)[:, None]
    topv = scores[rows, topi]

    # per-expert token lists
    idx_e = []
    val_e = []
    for e in range(E):
        tmask, kpos = np.nonzero(topi == e)
        idx_e.append(tmask)
        val_e.append(topv[tmask, kpos].astype(np.float32))
    max_cnt = max(len(i) for i in idx_e)

    # chunk geometry: NC chunks of ln <= 512 tokens (PSUM bank limit)
    NC = max(1, -(-max_cnt // 512))
    ln = _round_up(-(-max_cnt // NC), 4)
    C = NC * ln

    fold_gate = not b1.any()  # g*relu(u) == relu(g*u) only when b1 == 0
    s_e = w2.astype(np.float32).sum(axis=2)          # [E, H] row-sums
    b2s_e = b2.astype(np.float32).sum(axis=1)        # [E]

    nc = _build_program(D, H, ln, NC)

    in_maps = []
    for e in range(E):
        n_e = len(idx_e[e])
        xg = np.zeros((D, C), dtype=np.float32)
        if n_e:
            cols = xf[idx_e[e]].T
            if fold_gate:
                cols = cols * val_e[e][None, :]
            xg[:, :n_e] = cols
        # -> [P, KD*NC*ln], block (kd, c)
        xtg = np.ascontiguousarray(
            xg.reshape(KD, P, NC, ln).transpose(1, 0, 2, 3).reshape(P, KD * NC * ln)
        ).astype(bf16)
        # w1 -> [P, KD*H], block (g, kd) of GRP*P columns
        w1r = np.ascontiguousarray(
            w1[e]
            .astype(np.float32)
            .reshape(KD, P, MH // GRP, GRP * P)
            .transpose(1, 2, 0, 3)
            .reshape(P, KD * H)
        ).astype(bf16)
        sm = np.empty((P, 2 * MH + 1), dtype=np.float32)
        sm[:, 0:MH] = s_e[e].reshape(MH, P).T
        sm[:, MH : 2 * MH] = b1[e].astype(np.float32).reshape(MH, P).T
        sm[:, 2 * MH] = 1.0
        in_maps.append({"xtg": xtg, "w1": w1r, "sm": sm})

    res = run_bass_kernel_spmd(nc, in_maps, core_ids=list(range(N_CORES)))

    # --- combine: scatter-add per-(token, expert) scalars, then log_softmax ---
    s = np.zeros(T, dtype=np.float32)
    for e in range(E):
        n_e = len(idx_e[e])
        if n_e:
            z = res.results[e]["z"][0, :n_e].astype(np.float32)
            if fold_gate:
                s[idx_e[e]] += z
            else:
                s[idx_e[e]] += val_e[e] * z
    if b2s_e.any():
        for e in range(E):
            if len(idx_e[e]):
                s[idx_e[e]] += val_e[e] * b2s_e[e]

    sm = s.reshape(B, S)
    sm = sm - sm.max(axis=1, keepdims=True)
    out = sm - np.log(np.exp(sm).sum(axis=1, keepdims=True))
    return out.astype(np.float32)


# revision 17
# speedup vs baseline: 1.1716x; 1.0076x over previous
"""Expert-parallel MoE routing kernel for Trainium2 (8 NeuronCores).

Problem: top-k(=2) softmax-gated MoE FFN (relu), followed by
log_softmax(sum(moe_out, axis=-1)) over the sequence dim.

Key algebraic observation: the graded output is
    log_softmax_S( sum_d moe_out[t, d] )
and
    sum_d moe_out[t, :] = sum_e g[t,e] * (relu(x_t @ W1_e + b1_e) @ rowsum(W2_e) + sum(b2_e))
so the entire second expert matmul collapses to a matvec against
s_e = rowsum(W2_e), which the host computes once (a single pass over w2).
The gate values are folded into the dispatched tokens on the host
(g * relu(x@W1) == relu((g*x)@W1) for g > 0 when b1 == 0).

Per-core device program (core e owns expert e):
  PE : h_pre = xtg^T @ W1  (bf16 operands, 256 matmuls of ~276 cols)
  ACT: relu(h_pre + b1) per [128, ln] m-tile, PSUM -> SBUF
  DVE: acc += relu_h * s_m  (signed per-partition scale, fused mult-add)
  PE : z = ones^T @ acc     (final 128-partition reduction, 1 matmul/chunk)
Host gathers z per core, scatter-adds into [T], applies log_softmax.

HBM traffic per core is ~5.4 MB (bf16 w1 + bf16 gate-scaled tokens);
w2 (8 MB/core in the baseline) never touches the device.
"""

import os

import numpy as np

N_CORES = 8
P = 128
GRP = 2  # m-tiles per w1 column-group (one 256-col block per (group, kd))
NWARM = int(os.environ.get("MOE_NWARM", "10"))


def _round_up(v, m):
    return ((v + m - 1) // m) * m


_BUILD_CACHE = {}


def _build_program(D, H, ln, NC):
    """Trace + compile the single-core program (SPMD across 8 cores).

    Per-core inputs:
      xtg [P, KD*NC*ln] bf16  gate-scaled gathered tokens; block (kd, c) at
                              cols (kd*NC+c)*ln is xg[kd*P:(kd+1)*P, c*ln:(c+1)*ln]
      w1  [P, KD*H]     bf16  expert's first-layer weight; block (g, kd) at
                              cols (g*KD+kd)*GP is w1[kd*P:(kd+1)*P, g*GP:(g+1)*GP]
      sm  [P, 2*MH+1]   f32   cols [0:MH) = w2 row-sums (col m = s[m*P:(m+1)*P]),
                              [MH:2MH) = b1 tiled the same way, [2MH] = ones
    Output:
      z [1, NC*ln] f32  z[c] = sum_h s_h * relu(x_c @ w1_h + b1_h)
    """
    key = (D, H, ln, NC)
    if key in _BUILD_CACHE:
        return _BUILD_CACHE[key]

    import concourse.tile as tile
    from concourse import bacc, mybir

    f32 = mybir.dt.float32
    f32r = mybir.dt.float32r
    bf16 = mybir.dt.bfloat16
    KD = D // P   # k-tiles over D
    MH = H // P   # m-tiles over H
    NG = MH // GRP  # w1 column groups
    GP = GRP * P  # columns per w1 block

    nc = bacc.Bacc("TRN2", target_bir_lowering=False, debug=False)
    xtg_d = nc.dram_tensor("xtg", [P, KD * NC * ln], bf16, kind="ExternalInput").ap()
    w1_d = nc.dram_tensor("w1", [P, KD * H], bf16, kind="ExternalInput").ap()
    sm_d = nc.dram_tensor("sm", [P, 2 * MH + 1], f32, kind="ExternalInput").ap()
    z_d = nc.dram_tensor("z", [1, NC * ln], f32, kind="ExternalOutput").ap()

    with tile.TileContext(nc) as tc:
        with (
            tc.tile_pool(name="persist", bufs=1) as persist,
            tc.tile_pool(name="ht", bufs=6) as htp,
            tc.tile_pool(name="psum_h", bufs=6, space="PSUM") as psum_h,
            tc.tile_pool(name="psum_z", bufs=2, space="PSUM") as psum_z,
        ):
            # --- small loads first: w2 row-sums / b1 / ones ---
            sm_sb = persist.tile([P, 2 * MH + 1], f32)
            nc.sync.dma_start(out=sm_sb[:], in_=sm_d[:])
            w2s = sm_sb[:, 0:MH]
            b1t = sm_sb[:, MH : 2 * MH]
            ones = persist.tile([P, 1], f32r)
            nc.vector.tensor_copy(out=ones[:], in_=sm_sb[:, 2 * MH : 2 * MH + 1])

            # acc tiles — f32r so the final PE matvec accepts them; two
            # independent accumulation chains (even/odd m) per chunk halve
            # the serial DVE tail.  g == 0 writes them fresh (no memset:
            # walrus rejects f32r memset).
            acc = [
                [
                    persist.tile([P, ln], f32r, tag=f"acc{c}_{p}", name=f"acc{c}_{p}")
                    for p in range(GRP)
                ]
                for c in range(NC)
            ]

            # --- weights + activations, ordered for earliest PE start:
            # group 0's w1 kd-pieces interleaved with the xtg kd-pieces
            # (group 0 needs both; later groups only need their w1) ---
            xtg_sb = persist.tile([P, KD * NC * ln], bf16)
            w1_sb = persist.tile([P, KD * H], bf16)

            def xtg_load(kd0, nkd):
                sl = slice(kd0 * NC * ln, (kd0 + nkd) * NC * ln)
                nc.sync.dma_start(out=xtg_sb[:, sl], in_=xtg_d[:, sl])

            def w1_load(g, kd0, nkd):
                sl = slice((g * KD + kd0) * GP, (g * KD + kd0 + nkd) * GP)
                nc.sync.dma_start(out=w1_sb[:, sl], in_=w1_d[:, sl])

            # PE warmup: ~4.6us of dependency-free matmuls on scratch data
            # run during the fixed ~7us program prologue, flipping the HAM
            # clock-gate to 2.4GHz before the real stream starts (saves the
            # ~4us half-clock ramp the stream would otherwise pay)
            if NWARM:
                warm = persist.tile([P, ln], bf16, tag="warm", name="warm")
                nc.vector.memset(warm[:], 0.0)
                pwarm = psum_h.tile([P, ln], f32, tag="psh", name="psh")
                for i in range(NWARM):
                    nc.tensor.matmul(
                        pwarm[:],
                        warm[:, 0:P],
                        warm[:],
                        start=(i == 0),
                        stop=(i == NWARM - 1),
                        skip_group_check=True,
                    )

            # each dma_start costs ~650ns of serial HWDGE issue time and
            # ~1.5-2us completion latency, and transfers drain FIFO; lead
            # with two small pieces so the first matmuls' data lands early,
            # then stream the rest in big pieces
            h = KD // 2
            xtg_load(0, 1)
            w1_load(0, 0, 1)
            xtg_load(1, h - 1)
            w1_load(0, 1, h - 1)
            xtg_load(h, KD - h)
            w1_load(0, h, KD - h)
            w1_load(1, 0, h)
            w1_load(1, h, KD - h)
            for g in range(2, NG):
                w1_load(g, 0, KD)

            # --- mm1 + relu + scaled accumulate, group-major (chunk inner
            # so the w1 stream paces 2x ahead of the PE's consumption) ---
            for g in range(NG):
                pss = [
                    [
                        psum_h.tile([P, ln], f32, tag="psh", name="psh")
                        for _ in range(NC)
                    ]
                    for _ in range(GRP)
                ]
                for kd in range(KD):
                    base = (g * KD + kd) * GP
                    for mi in range(GRP):
                        for c in range(NC):
                            nc.tensor.matmul(
                                pss[mi][c][:],
                                w1_sb[:, base + mi * P : base + (mi + 1) * P],
                                xtg_sb[:, (kd * NC + c) * ln : (kd * NC + c + 1) * ln],
                                start=(kd == 0),
                                stop=(kd == KD - 1),
                                skip_group_check=True,
                            )
                for c in range(NC):  # chunk-major so chunk 0's output path
                    for mi in range(GRP):  # overlaps chunk 1's tail
                        m = g * GRP + mi
                        ht = htp.tile([P, ln], f32, tag="ht", name="ht")
                        nc.scalar.activation(
                            ht[:],
                            pss[mi][c][:],
                            mybir.ActivationFunctionType.Relu,
                            bias=b1t[:, m : m + 1],
                        )
                        if g == 0:
                            nc.vector.tensor_scalar(
                                out=acc[c][mi][:],
                                in0=ht[:],
                                scalar1=w2s[:, m : m + 1],
                                scalar2=None,
                                op0=mybir.AluOpType.mult,
                            )
                        else:
                            nc.vector.scalar_tensor_tensor(
                                out=acc[c][mi][:],
                                in0=ht[:],
                                scalar=w2s[:, m : m + 1],
                                in1=acc[c][mi][:],
                                op0=mybir.AluOpType.mult,
                                op1=mybir.AluOpType.add,
                            )

            # --- final partition reduction + output (per-chunk DMA so chunk
            # 0's output overlaps chunk 1's accumulate tail) ---
            z_sb = persist.tile([1, NC * ln], f32)
            for c in range(NC):
                pz = psum_z.tile([1, ln], f32, tag="psz", name="psz")
                for p in range(GRP):
                    nc.tensor.matmul(
                        pz[:],
                        ones[:],
                        acc[c][p][:],
                        start=(p == 0),
                        stop=(p == GRP - 1),
                        skip_group_check=True,
                    )
                sl = slice(c * ln, (c + 1) * ln)
                nc.scalar.activation(
                    z_sb[:, sl],
                    pz[:],
                    mybir.ActivationFunctionType.Copy,
                    bias=0.0,
                )
                nc.sync.dma_start(out=z_d[:, sl], in_=z_sb[:, sl])

    nc.compile()
    _BUILD_CACHE[key] = nc
    return nc


def kernel(x, wg, w1, b1, w2, b2, k):
    import ml_dtypes
    from concourse.bass_utils import run_bass_kernel_spmd

    bf16 = ml_dtypes.bfloat16
    x = np.asarray(x)
    wg = np.asarray(wg)
    w1 = np.asarray(w1)
    b1 = np.asarray(b1)
    w2 = np.asarray(w2)
    b2 = np.asarray(b2)
    k = int(k)

    B, S, D = x.shape
    E = wg.shape[1]
    H = w1.shape[2]
    T = B * S
    KD = D // P
    MH = H // P
    assert E == N_CORES, f"expert-parallel layout assumes E == 8, got {E}"
    assert D % P == 0 and H % P == 0 and MH % GRP == 0, (D, H)

    xf = np.ascontiguousarray(x.reshape(T, D), dtype=np.float32)

    # --- gate + top-k routing (host; needed to build the dispatch shards) ---
    logits = xf @ wg.astype(np.float32)
    logits -= logits.max(axis=1, keepdims=True)
    np.exp(logits, out=logits)
    scores = logits / logits.sum(axis=1, keepdims=True)
    if k >= E:
        topi = np.broadcast_to(np.arange(E, dtype=np.int64), (T, E))
    else:
        topi = np.argpartition(-scores, k, axis=1)[:, :k]
    rows = np.arange(T)[:, None]
    topv = scores[rows, topi]

    # per-expert token lists
    idx_e = []
    val_e = []
    for e in range(E):
        tmask, kpos = np.nonzero(topi == e)
        idx_e.append(tmask)
        val_e.append(topv[tmask, kpos].astype(np.float32))
    max_cnt = max(len(i) for i in idx_e)

    # chunk geometry: NC chunks of ln <= 512 tokens (PSUM bank limit)
    NC = max(1, -(-max_cnt // 512))
    ln = _round_up(-(-max_cnt // NC), 4)
    C = NC * ln

    fold_gate = not b1.any()  # g*relu(u) == relu(g*u) only when b1 == 0
    s_e = w2.astype(np.float32).sum(axis=2)          # [E, H] row-sums
    b2s_e = b2.astype(np.float32).sum(axis=1)        # [E]

    nc = _build_program(D, H, ln, NC)

    in_maps = []
    for e in range(E):
        n_e = len(idx_e[e])
        xg = np.zeros((D, C), dtype=np.float32)
        if n_e:
            cols = xf[idx_e[e]].T
            if fold_gate:
                cols = cols * val_e[e][None, :]
            xg[:, :n_e] = cols
        # -> [P, KD*NC*ln], block (kd, c)
        xtg = np.ascontiguousarray(
            xg.reshape(KD, P, NC, ln).transpose(1, 0, 2, 3).reshape(P, KD * NC * ln)
        ).astype(bf16)
        # w1 -> [P, KD*H], block (g, kd) of GRP*P columns
        w1r = np.ascontiguousarray(
            w1[e]
            .astype(np.float32)
            .reshape(KD, P, MH // GRP, GRP * P)
            .transpose(1, 2, 0, 3)
            .reshape(P, KD * H)
        ).astype(bf16)
        sm = np.empty((P, 2 * MH + 1), dtype=np.float32)
        sm[:, 0:MH] = s_e[e].reshape(MH, P).T
        sm[:, MH : 2 * MH] = b1[e].astype(np.float32).reshape(MH, P).T
        sm[:, 2 * MH] = 1.0
        in_maps.append({"xtg": xtg, "w1": w1r, "sm": sm})

    res = run_bass_kernel_spmd(nc, in_maps, core_ids=list(range(N_CORES)))

    # --- combine: scatter-add per-(token, expert) scalars, then log_softmax ---
    s = np.zeros(T, dtype=np.float32)
    for e in range(E):
        n_e = len(idx_e[e])
        if n_e:
            z = res.results[e]["z"][0, :n_e].astype(np.float32)
            if fold_gate:
                s[idx_e[e]] += z
            else:
                s[idx_e[e]] += val_e[e] * z
    if b2s_e.any():
        for e in range(E):
            if len(idx_e[e]):
                s[idx_e[e]] += val_e[e] * b2s_e[e]

    sm = s.reshape(B, S)
    sm = sm - sm.max(axis=1, keepdims=True)
    out = sm - np.log(np.exp(sm).sum(axis=1, keepdims=True))
    return out.astype(np.float32)
